# revision 28
# baseline (speedup 1.0000x reference)
import sys
if '/opt/trn_rl_repo' not in sys.path:
    sys.path.insert(0, '/opt/trn_rl_repo')
import numpy as np
import ml_dtypes

import concourse.bass as bass
import concourse.bacc as bacc
import concourse.tile as tile
from concourse import mybir
from concourse import bass_utils

f32 = mybir.dt.float32
f32r = mybir.dt.float32r
f8 = mybir.dt.float8e3          # e3m4: range +-15.5, 4 mantissa bits
f8np = ml_dtypes.float8_e3m4
FX = mybir.ActivationFunctionType
ALU = mybir.AluOpType
AX = mybir.AxisListType

B, D, H, DH = 256, 256, 8, 32
NCORES = 8
BC = B // NCORES          # 32 batches per core
LC = 1024                 # self-attn KV cache length
NA = 2048                 # cross-attn key count (raw)
NAC = 1152                # cross-attn keys after mask compaction (max real = 1105)
KT_S = LC // 128          # 8 key tiles (self)
KT_A = NAC // 128         # 9 key tiles (cross, compacted)
SCALE = 1.0 / float(np.sqrt(DH))
EPS = 1e-5
GB = 4                    # batches per DMA / unmix group
XS = 2 * LC + KT_S * 256      # 4096 bytes/partition per batch, self
XC = 2 * NAC + KT_A * 256     # 4608 bytes/partition per batch, cross

import os
ABLATE = os.environ.get('BASS_ABLATE', '')   # '', 'dma', 'noscore', 'nov'

WNAMES = ['wq_s', 'wk_s', 'wv_s', 'w0_s', 'wq_a', 'w0_a', 'w1', 'w2']
BNAMES = ['bq_s', 'bk_s', 'bv_s', 'b0_s', 'bq_a', 'b0_a', 'b1', 'b2']
LNAMES = ['ln1_g', 'ln1_b', 'ln2_g', 'ln2_b', 'ln3_g', 'ln3_b']


def _declare_dram(nc):
    dr = {}
    dr['h_t'] = nc.dram_tensor('h_t', [BC, 1, D], f32, kind='ExternalInput')
    # Host-packed fp8 per batch: K^T region (2 chunks x nt x 128 cols, partition=d)
    # then V region (nt tiles x 256 cols, partition=key). Masked/padded keys have
    # V rows zeroed (so they drop out of numerator and denominator).
    dr['KV_att'] = nc.dram_tensor('KV_att', [BC, 128, XC], f8, kind='ExternalInput')
    dr['KV_cache'] = nc.dram_tensor('KV_cache', [BC, 128, XS], f8, kind='ExternalInput')
    # keep-mask (1.0 = keep) per compacted key slot, fp8, for the denominator matmul
    dr['notm8'] = nc.dram_tensor('notm8', [128, KT_A, BC], f8, kind='ExternalInput')
    dr['ident'] = nc.dram_tensor('ident', [128, 128], f32, kind='ExternalInput')
    # unmix indicator: E[h, c, i] = 1 iff head h == 4c + i//32 (for inv broadcast)
    dr['unmix_E'] = nc.dram_tensor('unmix_E', [8, 2, 128], f32, kind='ExternalInput')
    for n in WNAMES:
        dr[n] = nc.dram_tensor(n, [D, D], f32r, kind='ExternalInput')
    for n in BNAMES + LNAMES:
        dr[n] = nc.dram_tensor(n, [D], f32, kind='ExternalInput')
    dr['out'] = nc.dram_tensor('out', [BC, D], f32, kind='ExternalOutput')
    return dr


def _build():
    nc = bacc.Bacc()
    dr = _declare_dram(nc)
    out = dr.pop('out')
    with tile.TileContext(nc) as tc:
        _emit(nc, tc, dr, out)
    nc.compile()
    return nc


def _emit(nc, tc, dr, out_dram):
    import contextlib
    ctx = contextlib.ExitStack()
    with ctx:
        cload = ctx.enter_context(tc.tile_pool(name='cload', bufs=2))
        const = ctx.enter_context(tc.tile_pool(name='const', bufs=1))
        kvs_p = ctx.enter_context(tc.tile_pool(name='kvs', bufs=3))
        kva_p = ctx.enter_context(tc.tile_pool(name='kva', bufs=4))
        wx_p = ctx.enter_context(tc.tile_pool(name='wx', bufs=3))
        dn_p = ctx.enter_context(tc.tile_pool(name='dn', bufs=3))
        tr_ps = ctx.enter_context(tc.tile_pool(name='trps', bufs=1, space='PSUM'))
        sc_ps = ctx.enter_context(tc.tile_pool(name='scps', bufs=3, space='PSUM'))
        at_ps = ctx.enter_context(tc.tile_pool(name='atps', bufs=2, space='PSUM'))
        ln_ps = ctx.enter_context(tc.tile_pool(name='lnps', bufs=1, space='PSUM'))
        gb_ps = ctx.enter_context(tc.tile_pool(name='gbps', bufs=1, space='PSUM'))

        garb = gb_ps.tile([1, 1], f32, tag='garb')
        last_act = [None]

        def pe_absorb(*aps):
            # PE matmul/transpose can carry only ONE sem wait in its LW slot.
            # Before a matmul whose deps span several procs, emit 1x1
            # self-matmuls so the PE observes those sems here instead.
            for a in aps:
                if a is None:
                    continue
                e = a[tuple(slice(0, 1) for _ in range(len(a.shape)))]
                if e.dtype == f32r:
                    e = e.bitcast(f32)
                nc.tensor.matmul(garb[:, :], e, e, start=True, stop=True,
                                 skip_group_check=True)

        # ---------- persistent loads ----------
        ident = cload.tile([128, 128], f32, tag='ident')
        nc.gpsimd.dma_start(out=ident, in_=dr['ident'][:, :])
        pe_absorb(ident)
        ht = cload.tile([BC, D], f32, tag='ht')
        nc.gpsimd.dma_start(out=ht, in_=dr['h_t'][:, 0, :])
        pe_absorb(ht)
        epst = const.tile([BC, 1], f32, tag='epst')
        nc.vector.memset(epst, EPS)
        ones8 = const.tile([128, 1], f8, tag='ones8')
        nc.vector.memset(ones8, 1.0)
        ones32 = const.tile([128, 1], f32, tag='ones32')
        nc.vector.memset(ones32, 1.0)
        unmixE = cload.tile([8, 2, 128], f32, tag='unmixE')
        nc.gpsimd.dma_start(out=unmixE, in_=dr['unmix_E'][:, :, :])
        pe_absorb(unmixE)

        wsb = {}
        for n in WNAMES:
            wsb[n] = cload.tile([128, 2, D], f32r, tag='w_' + n, name='w_' + n)
            nc.gpsimd.dma_start(out=wsb[n], in_=dr[n][:, :].rearrange('(t p) j -> p t j', p=128))
        vsb = {}
        for n in BNAMES + LNAMES:
            vsb[n] = cload.tile([BC, D], f32, tag='v_' + n, name='v_' + n)
            nc.gpsimd.dma_start(out=vsb[n], in_=dr[n][:].unsqueeze(0).to_broadcast([BC, D]))

        notm8 = cload.tile([128, KT_A, BC], f8, tag='notm8')
        nc.gpsimd.dma_start(out=notm8, in_=dr['notm8'][:, :, :])

        # ---------- helpers ----------
        def transpose_128(dst, src, cols):
            # src [rows<=128, cols<=128] SBUF f32 -> dst [cols, rows] via PE transpose
            rows = src.shape[0]
            ps = tr_ps.tile([128, 128], f32, tag='trps')
            nc.tensor.transpose(ps[0:cols, 0:rows], src, ident[0:rows, 0:rows])
            nc.vector.tensor_copy(out=dst, in_=ps[0:cols, 0:rows])

        def make_T(src_f32, tagname):
            # src [BC, D] -> [128, 2, BC] f32r transposed halves
            dstT = const.tile([128, 2, BC], f32r, tag=tagname, name=tagname)
            for t in range(2):
                transpose_128(dstT[:, t, :], src_f32[:, 128 * t:128 * (t + 1)], 128)
            return dstT

        def linear_psum(srcT_list, wname):
            # sum_t sum_s srcT.T @ W  -> psum [BC, D]
            ps = ln_ps.tile([BC, D], f32, tag='lnps')
            pe_absorb(wsb[wname])
            n_mm = 2 * len(srcT_list)
            i = 0
            for srcT in srcT_list:
                for t in range(2):
                    nc.tensor.matmul(ps[:, :], srcT[:, t, :], wsb[wname][:, t, :],
                                     start=(i == 0), stop=(i == n_mm - 1))
                    i += 1
            return ps

        def layernorm(dst, src, gname, bname, tagp):
            stats = const.tile([BC, 6], f32, tag=tagp + '_st', name=tagp + '_st')
            nc.vector.bn_stats(out=stats, in_=src)
            mv = const.tile([BC, 2], f32, tag=tagp + '_mv', name=tagp + '_mv')
            nc.vector.bn_aggr(out=mv, in_=stats)
            sd = const.tile([BC, 1], f32, tag=tagp + '_sd', name=tagp + '_sd')
            nc.scalar.activation(out=sd, in_=mv[:, 1:2], func=FX.Sqrt,
                                 bias=epst[:, :], scale=1.0)
            rstd = const.tile([BC, 1], f32, tag=tagp + '_rs', name=tagp + '_rs')
            nc.vector.reciprocal(out=rstd, in_=sd)
            nc.vector.tensor_scalar(out=dst, in0=src, scalar1=mv[:, 0:1], scalar2=rstd,
                                    op0=ALU.subtract, op1=ALU.mult)
            nc.vector.tensor_mul(dst, dst, vsb[gname])
            nc.vector.tensor_add(dst, dst, vsb[bname])

        def build_qblk(qsrc_f32, tagp):
            # -> [128, 2, BC, H] fp8, block-diagonal per head (zeros elsewhere)
            qT = make_T(qsrc_f32, tagp + '_qT')
            qb = const.tile([128, 2, BC, H], f8, tag=tagp + '_qb', name=tagp + '_qb')
            nc.vector.memset(qb, 0.0)
            for t in range(2):
                for hh in range(4):
                    h = 4 * t + hh
                    nc.vector.tensor_copy(out=qb[32 * hh:32 * (hh + 1), t, :, h],
                                          in_=qT[32 * hh:32 * (hh + 1), t, :])
            return qb

        # ---------- qkv for self-attn ----------
        htT = make_T(ht, 'htT')
        qkv = {}
        for nm, wn, bn in (('q', 'wq_s', 'bq_s'), ('k', 'wk_s', 'bk_s'), ('v', 'wv_s', 'bv_s')):
            ps = linear_psum([htT], wn)
            qkv[nm] = const.tile([BC, D], f32, tag='qkv_' + nm, name='qkv_' + nm)
            nc.vector.tensor_add(qkv[nm], ps, vsb[bn])

        qblk_s = build_qblk(qkv['q'], 'self')

        # new-key (appended k/v) terms, all-batch
        qk = const.tile([BC, D], f32, tag='qk')
        nc.vector.tensor_mul(qk, qkv['q'], qkv['k'])
        s_new = const.tile([BC, H], f32, tag='s_new')
        nc.vector.reduce_sum(out=s_new, in_=qk.rearrange('p (g s) -> p g s', g=H), axis=AX.X)
        w_new = const.tile([BC, H], f32, tag='w_new')
        nc.scalar.activation(out=w_new, in_=s_new, func=FX.Exp, scale=SCALE)
        w_newT = const.tile([H, BC], f32, tag='w_newT')
        pe_absorb(w_new)
        transpose_128(w_newT, w_new, H)

        # ---------- attention inner loop ----------
        # scores:  sps[key, t, h] = sum_d K^T[d, key] * qblk[d, h]  (2 chunk MMs)
        # weights: wex = exp(scale * sps)  (fp8)
        # numer:   psum4[d, c, 8jb+h] += V_tile[key, d] * wex[key, h]  (V stationary)
        # denom:   psum4[h, 64+jb] += wex[key, h] * keep[key]
        scr = const.tile([128, 16], f8, tag='scr')
        wex0 = None
        if ABLATE == 'noscore':
            wex0 = const.tile([128, KT_A, H], f8, tag='wex0')
            nc.vector.memset(wex0, 0.5)

        # ---------- attention inner loop (software-pipelined) ----------
        # scores:  sps[key, t, h] = sum_d K^T[d, key] * qblk[d, h]  (2 chunk MMs)
        # weights: wex = exp(scale * sps)  (fp8, ACT)
        # pipeline: scores/exp of instance b+1 issue before V-matmuls of b,
        # so the PE never stalls on the ACT exp round-trip.
        # numer:   psum4[d, c, 8jb+h] += V_tile[key, d] * wex[key, h]  (V stationary,
        #          chunk-contiguous accumulation passes)
        # denom:   DVE tile-reduce of wex (masked for cross) + one matmul
        def attention(qblk, n_tiles, KV_dram, attT_dst, dnmix, masked):
            ksz = n_tiles * 128           # K^T bytes per chunk per partition
            voff = 2 * ksz                # V region offset
            tot = voff + n_tiles * 256
            ngroups = BC // GB
            kvps = [None] * ngroups
            psum4s = [None] * ngroups
            pend = {}
            if ABLATE:
                nc.vector.memset(attT_dst.bitcast(f32), 0.01)
                nc.vector.memset(dnmix[0], 1.0)

            kv_p = kva_p if masked else kvs_p
            def issue_dma(g):
                kvps[g] = kv_p.tile([128, GB, tot], f8, tag='kv%d' % tot, name='kvp%d' % g)
                for q in range(GB):
                    nc.sync.dma_start(out=kvps[g][:, q, :],
                                      in_=KV_dram[GB * g + q])
                if ABLATE == 'dma':
                    nc.vector.tensor_copy(out=scr[:, 0:8], in_=kvps[g][:, 0, 0:8])

            def scores_exp(bi):
                g, jb = divmod(bi, GB)
                kv = kvps[g][:, jb, :]
                if ABLATE == 'noscore':
                    pend[bi] = (wex0, kv)
                    return
                sps = sc_ps.tile([128, KT_A, H], f32, tag='scps')
                if ABLATE == 'halfs':
                    for t in range(n_tiles):
                        nc.tensor.matmul(sps[:, t, :], kv[:, 128 * t:128 * (t + 1)],
                                         qblk[:, 0, GB * g + jb, :], start=True, stop=True)
                else:
                    for t in range(n_tiles):
                        nc.tensor.matmul(sps[:, t, :], kv[:, 128 * t:128 * (t + 1)],
                                         qblk[:, 0, GB * g + jb, :], start=True, stop=False)
                        nc.tensor.matmul(sps[:, t, :], kv[:, ksz + 128 * t:ksz + 128 * (t + 1)],
                                         qblk[:, 1, GB * g + jb, :], start=False, stop=True)
                wex = wx_p.tile([128, KT_A, H], f8, tag='wex')
                nc.scalar.activation(out=wex[:, 0:n_tiles, :], in_=sps[:, 0:n_tiles, :],
                                     func=FX.Exp, scale=SCALE)
                pend[bi] = (wex, kv)

            def v_dn(bi):
                g, jb = divmod(bi, GB)
                b = bi
                wex, kv = pend.pop(bi)
                if ABLATE == 'nov':
                    nc.vector.tensor_copy(out=scr[:, 8:16], in_=wex[:, 0, 0:8])
                    return
                if jb == 0:
                    psum4s[g] = at_ps.tile([128, 72], f32, tag='at72', name='at72_%d' % g)
                psum4 = psum4s[g]
                # numerator (V stationary), chunk-contiguous accumulation passes
                for c in range(1 if ABLATE == 'halfv' else 2):
                    for t in range(n_tiles):
                        vt = kv[:, voff + 256 * t + 128 * c:voff + 256 * t + 128 * (c + 1)]
                        nc.tensor.matmul(psum4[:, 32 * c + 8 * jb:32 * c + 8 * jb + 8],
                                         vt, wex[:, t, :],
                                         start=(t == 0), stop=(t == n_tiles - 1))
                # denominator: DVE tile-reduce (masked for cross) + one matmul
                dnp = dn_p.tile([128, 8], f32, tag='dnp')
                if masked:
                    wexm = dn_p.tile([128, KT_A, 8], f32, tag='wexm')
                    nc.vector.tensor_tensor(
                        out=wexm[:, 0:n_tiles, :], in0=wex[:, 0:n_tiles, :],
                        in1=notm8[:, 0:n_tiles, b:b + 1].broadcast_to([128, n_tiles, H]),
                        op=ALU.mult)
                    red_in = wexm[:, 0:n_tiles, :]
                else:
                    red_in = wex[:, 0:n_tiles, :]
                nc.vector.reduce_sum(out=dnp, in_=red_in.rearrange('p t h -> p h t'),
                                     axis=AX.X)
                nc.tensor.matmul(psum4[0:8, 64 + jb:65 + jb], dnp, ones32,
                                 start=True, stop=True)
                if jb == GB - 1:
                    epilogue(g, psum4)

            def epilogue(g, psum4):
                # denominators + unmix copies (DVE)
                if dnmix[1] is None:
                    nc.vector.tensor_copy(out=dnmix[0][:, GB * g:GB * (g + 1)],
                                          in_=psum4[0:8, 64:68])
                else:
                    nc.vector.tensor_add(dnmix[0][:, GB * g:GB * (g + 1)],
                                         psum4[0:8, 64:68],
                                         dnmix[1][:, GB * g:GB * (g + 1)])
                pview = psum4[:, 0:64].rearrange('p (c jb h) -> p c h jb', c=2, jb=GB)
                for c in range(2):
                    for k in range(4):
                        h = 4 * c + k
                        nc.vector.tensor_copy(
                            out=attT_dst[32 * k:32 * (k + 1), c, GB * g:GB * (g + 1)],
                            in_=pview[32 * k:32 * (k + 1), c, h, :])

            issue_dma(0)
            if ngroups > 1:
                issue_dma(1)
            if ABLATE == 'dma':
                for g in range(2, ngroups):
                    issue_dma(g)
                return
            LOOK = 2   # instances of scores lookahead over the V/denominator stage
            for bi in range(BC):
                g, jb = divmod(bi, GB)
                if jb == 0:
                    if bi >= LOOK:
                        v_dn(bi - LOOK)   # before the DMA wait
                    if g + 2 < ngroups:
                        issue_dma(g + 2)
                else:
                    if bi >= LOOK:
                        v_dn(bi - LOOK)
                pe_absorb(kvps[g][0:1, jb:jb + 1, 0:1])
                scores_exp(bi)
            for bi in range(BC - LOOK, BC):
                v_dn(bi)

        def apply_inv(attT_dst, dnmix_t, tagp):
            # attT *= 1/denom, broadcast per (head, batch) via indicator matmul
            inv = const.tile([H, BC], f32, tag=tagp + '_inv', name=tagp + '_inv')
            nc.vector.reciprocal(out=inv, in_=dnmix_t)
            ips = tr_ps.tile([128, 128], f32, tag='trps')
            ipv = ips[:, 0:2 * BC].rearrange('p (c b) -> p c b', c=2)
            pe_absorb(inv)
            for c in range(2):
                nc.tensor.matmul(ipv[:, c, :], unmixE[:, c, :], inv,
                                 start=True, stop=True)
            invx = const.tile([128, 2, BC], f32, tag=tagp + '_invx', name=tagp + '_invx')
            nc.vector.tensor_copy(out=invx, in_=ipv)
            nc.vector.tensor_mul(attT_dst, attT_dst.bitcast(f32), invx)

        # ---------- self attention ----------
        attT_s = const.tile([128, 2, BC], f32r, tag='attT_s')
        dnmix_s = const.tile([H, BC], f32, tag='dnmix_s')
        attention(qblk_s, KT_S, dr['KV_cache'], attT_s, (dnmix_s, w_newT), False)

        # new-key numerator: nv = v * w_new (batch layout), transpose, add to attT
        nv = const.tile([BC, D], f32, tag='nv')
        nc.vector.tensor_tensor(out=nv.rearrange('p (g s) -> p g s', g=H),
                                in0=qkv['v'].rearrange('p (g s) -> p g s', g=H),
                                in1=w_new.unsqueeze(2).broadcast_to([BC, H, DH]),
                                op=ALU.mult)
        nvT = make_T(nv, 'nvT')
        nc.vector.tensor_add(attT_s, attT_s.bitcast(f32), nvT.bitcast(f32))
        apply_inv(attT_s, dnmix_s, 'self')

        # h1 = LN1(ht + att_self @ w0_s + b0_s)
        ps = linear_psum([attT_s], 'w0_s')
        h1p = const.tile([BC, D], f32, tag='h1p')
        nc.vector.tensor_add(h1p, ps, vsb['b0_s'])
        nc.vector.tensor_add(h1p, h1p, ht)
        h1 = const.tile([BC, D], f32, tag='h1')
        layernorm(h1, h1p, 'ln1_g', 'ln1_b', 'ln1')

        # ---------- cross attention ----------
        h1T = make_T(h1, 'h1T')
        psq = linear_psum([h1T], 'wq_a')
        qa = const.tile([BC, D], f32, tag='qa')
        nc.vector.tensor_add(qa, psq, vsb['bq_a'])
        qblk_a = build_qblk(qa, 'cross')

        attT_a = const.tile([128, 2, BC], f32r, tag='attT_a')
        dnmix_a = const.tile([H, BC], f32, tag='dnmix_a')
        attention(qblk_a, KT_A, dr['KV_att'], attT_a, (dnmix_a, None), True)
        apply_inv(attT_a, dnmix_a, 'cross')

        # h2 = LN2(h1 + att_cross @ w0_a + b0_a)
        ps2 = linear_psum([attT_a], 'w0_a')
        h2p = const.tile([BC, D], f32, tag='h2p')
        nc.vector.tensor_add(h2p, ps2, vsb['b0_a'])
        nc.vector.tensor_add(h2p, h2p, h1)
        h2 = const.tile([BC, D], f32, tag='h2')
        layernorm(h2, h2p, 'ln2_g', 'ln2_b', 'ln2')

        # ---------- MLP ----------
        h2T = make_T(h2, 'h2T')
        psm = linear_psum([h2T], 'w1')
        m1 = const.tile([BC, D], f32, tag='m1')
        nc.vector.tensor_add(m1, psm, vsb['b1'])
        m1r = const.tile([BC, D], f32, tag='m1r')
        nc.scalar.activation(out=m1r, in_=m1, func=FX.Relu, scale=1.0)
        pe_absorb(m1r)
        m1T = make_T(m1r, 'm1T')
        psm2 = linear_psum([m1T], 'w2')
        h3p = const.tile([BC, D], f32, tag='h3p')
        nc.vector.tensor_add(h3p, psm2, vsb['b2'])
        nc.vector.tensor_add(h3p, h3p, h2)
        outt = const.tile([BC, D], f32, tag='outt')
        layernorm(outt, h3p, 'ln3_g', 'ln3_b', 'ln3')
        nc.scalar.dma_start(out=out_dram[:, :], in_=outt)


_CACHE = {}


def _get_nc():
    if 'nc' not in _CACHE:
        _CACHE['nc'] = _build()
    return _CACHE['nc']


def _pack_kv(K, V):
    # K, V: [BC, n, D] float arrays (already compacted/zero-padded)
    # -> [BC, 128, 2*n + (n//128)*256] fp8
    # K^T region: col (c*n + 128*t + p), partition dd  = K[b, 128t+p, 128c+dd]
    # V region:   col (2n + 256*t + j), partition p    = V[b, 128t+p, j]
    n = K.shape[1]
    nt = n // 128
    k8 = K.astype(f8np).reshape(BC, nt, 128, 2, 128)             # [b, t, p, c, dd]
    kp = np.ascontiguousarray(k8.transpose(0, 4, 3, 1, 2))       # [b, dd, c, t, p]
    kp = kp.reshape(BC, 128, 2 * n)
    v8 = V.astype(f8np).reshape(BC, nt, 128, D)                  # [b, t, p, j]
    vp = np.ascontiguousarray(v8.transpose(0, 2, 1, 3)).reshape(BC, 128, nt * D)
    return np.concatenate([kp, vp], axis=2)


def _compact(K, V, mask):
    # keep only unmasked keys (order preserved), zero-pad to NAC
    Kc = np.zeros((BC, NAC, D), dtype=np.float32)
    Vc = np.zeros((BC, NAC, D), dtype=np.float32)
    keep = np.zeros((BC, NAC), dtype=np.float32)
    for b in range(BC):
        idx = np.nonzero(~mask[b])[0]
        cnt = len(idx)
        assert cnt <= NAC, f"unmasked count {cnt} exceeds capacity {NAC}"
        Kc[b, :cnt] = K[b, idx]
        Vc[b, :cnt] = V[b, idx]
        keep[b, :cnt] = 1.0
    return Kc, Vc, keep


def _make_in_maps(inputs):
    np_in = {k: np.asarray(v) for k, v in inputs.items()}
    ident = np.eye(128, dtype=np.float32)
    unmix_E = np.zeros((8, 2, 128), dtype=np.float32)
    for h in range(8):
        c, k = h // 4, h % 4
        unmix_E[h, c, 32 * k:32 * (k + 1)] = 1.0
    in_maps = []
    for cre in range(NCORES):
        sl = slice(cre * BC, (cre + 1) * BC)
        Kc, Vc, keep = _compact(np_in['K_att'][sl], np_in['V_att'][sl],
                                np_in['mask'][sl])
        # keep-mask in packed (p, t, b) order: slot index = 128t+p
        notm = keep.reshape(BC, KT_A, 128).transpose(2, 1, 0)
        im = {
            'h_t': np.ascontiguousarray(np_in['h_t'][sl]),
            'KV_att': _pack_kv(Kc, Vc),
            'KV_cache': _pack_kv(np_in['K_cache'][sl], np_in['V_cache'][sl]),
            'notm8': np.ascontiguousarray(notm).astype(f8np),
            'ident': ident,
            'unmix_E': unmix_E,
        }
        for n in WNAMES + BNAMES + LNAMES:
            im[n] = np.ascontiguousarray(np_in[n])
        in_maps.append(im)
    return in_maps


def run_on_device(inputs):
    nc = _get_nc()
    in_maps = _make_in_maps(inputs)
    res = bass_utils.run_bass_kernel_spmd(nc, in_maps, core_ids=list(range(NCORES)),
                                          trace=False)
    outs = [res.results[c]['out'] for c in range(NCORES)]
    return np.concatenate(outs, axis=0).astype(np.float32)


def kernel(**inputs):
    return run_on_device(inputs)


# revision 29
# speedup vs baseline: 1.0420x; 1.0420x over previous
import sys
if '/opt/trn_rl_repo' not in sys.path:
    sys.path.insert(0, '/opt/trn_rl_repo')
import numpy as np
import ml_dtypes

import concourse.bass as bass
import concourse.bacc as bacc
import concourse.tile as tile
from concourse import mybir
from concourse import bass_utils

f32 = mybir.dt.float32
f32r = mybir.dt.float32r
f8 = mybir.dt.float8e3          # e3m4: range +-15.5, 4 mantissa bits
f8np = ml_dtypes.float8_e3m4
FX = mybir.ActivationFunctionType
ALU = mybir.AluOpType
AX = mybir.AxisListType

B, D, H, DH = 256, 256, 8, 32
NCORES = 8
BC = B // NCORES          # 32 batches per core
LC = 1024                 # self-attn KV cache length
NA = 2048                 # cross-attn key count (raw)
NAC = 1152                # cross-attn keys after mask compaction (max real = 1105)
KT_S = LC // 128          # 8 key tiles (self)
KT_A = NAC // 128         # 9 key tiles (cross, compacted)
SCALE = 1.0 / float(np.sqrt(DH))
EPS = 1e-5
GB = 4                    # batches per DMA / unmix group
XS = 2 * LC + KT_S * 256      # 4096 bytes/partition per batch, self
XC = 2 * NAC + KT_A * 256     # 4608 bytes/partition per batch, cross

import os
ABLATE = os.environ.get('BASS_ABLATE', '')   # '', 'dma', 'noscore', 'nov'

WNAMES = ['wq_s', 'wk_s', 'wv_s', 'w0_s', 'wq_a', 'w0_a', 'w1', 'w2']
BNAMES = ['bq_s', 'bk_s', 'bv_s', 'b0_s', 'bq_a', 'b0_a', 'b1', 'b2']
LNAMES = ['ln1_g', 'ln1_b', 'ln2_g', 'ln2_b', 'ln3_g', 'ln3_b']


def _declare_dram(nc):
    dr = {}
    dr['h_t'] = nc.dram_tensor('h_t', [BC, 1, D], f32, kind='ExternalInput')
    # Host-packed fp8 per batch: K^T region (2 chunks x nt x 128 cols, partition=d)
    # then V region (nt tiles x 256 cols, partition=key). Masked/padded keys have
    # V rows zeroed (so they drop out of numerator and denominator).
    dr['KV_att'] = nc.dram_tensor('KV_att', [BC, 128, XC], f8, kind='ExternalInput')
    dr['KV_cache'] = nc.dram_tensor('KV_cache', [BC, 128, XS], f8, kind='ExternalInput')
    # keep-mask (1.0 = keep) per compacted key slot, fp8, for the denominator matmul
    dr['notm8'] = nc.dram_tensor('notm8', [128, KT_A, BC], f8, kind='ExternalInput')
    dr['ident'] = nc.dram_tensor('ident', [128, 128], f32, kind='ExternalInput')
    # unmix indicator: E[h, c, i] = 1 iff head h == 4c + i//32 (for inv broadcast)
    dr['unmix_E'] = nc.dram_tensor('unmix_E', [8, 2, 128], f32, kind='ExternalInput')
    for n in WNAMES:
        dr[n] = nc.dram_tensor(n, [D, D], f32r, kind='ExternalInput')
    for n in BNAMES + LNAMES:
        dr[n] = nc.dram_tensor(n, [D], f32, kind='ExternalInput')
    dr['out'] = nc.dram_tensor('out', [BC, D], f32, kind='ExternalOutput')
    return dr


def _build():
    nc = bacc.Bacc()
    dr = _declare_dram(nc)
    out = dr.pop('out')
    with tile.TileContext(nc) as tc:
        _emit(nc, tc, dr, out)
    nc.compile()
    return nc


def _emit(nc, tc, dr, out_dram):
    import contextlib
    ctx = contextlib.ExitStack()
    with ctx:
        cload = ctx.enter_context(tc.tile_pool(name='cload', bufs=2))
        const = ctx.enter_context(tc.tile_pool(name='const', bufs=1))
        kvs_p = ctx.enter_context(tc.tile_pool(name='kvs', bufs=3))
        kva_p = ctx.enter_context(tc.tile_pool(name='kva', bufs=4))
        wx_p = ctx.enter_context(tc.tile_pool(name='wx', bufs=3))
        dn_p = ctx.enter_context(tc.tile_pool(name='dn', bufs=3))
        tr_ps = ctx.enter_context(tc.tile_pool(name='trps', bufs=1, space='PSUM'))
        sc_ps = ctx.enter_context(tc.tile_pool(name='scps', bufs=3, space='PSUM'))
        at_ps = ctx.enter_context(tc.tile_pool(name='atps', bufs=2, space='PSUM'))
        ln_ps = ctx.enter_context(tc.tile_pool(name='lnps', bufs=1, space='PSUM'))
        gb_ps = ctx.enter_context(tc.tile_pool(name='gbps', bufs=1, space='PSUM'))

        garb = gb_ps.tile([1, 1], f32, tag='garb')
        last_act = [None]

        def pe_absorb(*aps):
            # PE matmul/transpose can carry only ONE sem wait in its LW slot.
            # Before a matmul whose deps span several procs, emit 1x1
            # self-matmuls so the PE observes those sems here instead.
            for a in aps:
                if a is None:
                    continue
                e = a[tuple(slice(0, 1) for _ in range(len(a.shape)))]
                if e.dtype == f32r:
                    e = e.bitcast(f32)
                nc.tensor.matmul(garb[:, :], e, e, start=True, stop=True,
                                 skip_group_check=True)

        # ---------- persistent loads ----------
        ident = cload.tile([128, 128], f32, tag='ident')
        nc.gpsimd.dma_start(out=ident, in_=dr['ident'][:, :])
        pe_absorb(ident)
        ht = cload.tile([BC, D], f32, tag='ht')
        nc.gpsimd.dma_start(out=ht, in_=dr['h_t'][:, 0, :])
        pe_absorb(ht)
        epst = const.tile([BC, 1], f32, tag='epst')
        nc.vector.memset(epst, EPS)
        ones8 = const.tile([128, 1], f8, tag='ones8')
        nc.vector.memset(ones8, 1.0)
        ones32 = const.tile([128, 1], f32, tag='ones32')
        nc.vector.memset(ones32, 1.0)
        unmixE = cload.tile([8, 2, 128], f32, tag='unmixE')
        nc.gpsimd.dma_start(out=unmixE, in_=dr['unmix_E'][:, :, :])
        pe_absorb(unmixE)

        wsb = {}
        for n in WNAMES:
            wsb[n] = cload.tile([128, 2, D], f32r, tag='w_' + n, name='w_' + n)
            nc.gpsimd.dma_start(out=wsb[n], in_=dr[n][:, :].rearrange('(t p) j -> p t j', p=128))
        vsb = {}
        for n in BNAMES + LNAMES:
            vsb[n] = cload.tile([BC, D], f32, tag='v_' + n, name='v_' + n)
            nc.gpsimd.dma_start(out=vsb[n], in_=dr[n][:].unsqueeze(0).to_broadcast([BC, D]))

        notm8 = cload.tile([128, KT_A, BC], f8, tag='notm8')
        nc.gpsimd.dma_start(out=notm8, in_=dr['notm8'][:, :, :])

        # ---------- helpers ----------
        def transpose_128(dst, src, cols):
            # src [rows<=128, cols<=128] SBUF f32 -> dst [cols, rows] via PE transpose
            rows = src.shape[0]
            ps = tr_ps.tile([128, 128], f32, tag='trps')
            nc.tensor.transpose(ps[0:cols, 0:rows], src, ident[0:rows, 0:rows])
            nc.vector.tensor_copy(out=dst, in_=ps[0:cols, 0:rows])

        def make_T(src_f32, tagname):
            # src [BC, D] -> [128, 2, BC] f32r transposed halves
            dstT = const.tile([128, 2, BC], f32r, tag=tagname, name=tagname)
            for t in range(2):
                transpose_128(dstT[:, t, :], src_f32[:, 128 * t:128 * (t + 1)], 128)
            return dstT

        def linear_psum(srcT_list, wname):
            # sum_t sum_s srcT.T @ W  -> psum [BC, D]
            ps = ln_ps.tile([BC, D], f32, tag='lnps')
            pe_absorb(wsb[wname])
            n_mm = 2 * len(srcT_list)
            i = 0
            for srcT in srcT_list:
                for t in range(2):
                    nc.tensor.matmul(ps[:, :], srcT[:, t, :], wsb[wname][:, t, :],
                                     start=(i == 0), stop=(i == n_mm - 1))
                    i += 1
            return ps

        def layernorm(dst, src, gname, bname, tagp):
            stats = const.tile([BC, 6], f32, tag=tagp + '_st', name=tagp + '_st')
            nc.vector.bn_stats(out=stats, in_=src)
            mv = const.tile([BC, 2], f32, tag=tagp + '_mv', name=tagp + '_mv')
            nc.vector.bn_aggr(out=mv, in_=stats)
            sd = const.tile([BC, 1], f32, tag=tagp + '_sd', name=tagp + '_sd')
            nc.scalar.activation(out=sd, in_=mv[:, 1:2], func=FX.Sqrt,
                                 bias=epst[:, :], scale=1.0)
            rstd = const.tile([BC, 1], f32, tag=tagp + '_rs', name=tagp + '_rs')
            nc.vector.reciprocal(out=rstd, in_=sd)
            nc.vector.tensor_scalar(out=dst, in0=src, scalar1=mv[:, 0:1], scalar2=rstd,
                                    op0=ALU.subtract, op1=ALU.mult)
            nc.vector.tensor_mul(dst, dst, vsb[gname])
            nc.vector.tensor_add(dst, dst, vsb[bname])

        def build_qblk(qsrc_f32, tagp):
            # -> [128, 2, BC, H] fp8, block-diagonal per head (zeros elsewhere)
            qT = make_T(qsrc_f32, tagp + '_qT')
            qb = const.tile([128, 2, BC, H], f8, tag=tagp + '_qb', name=tagp + '_qb')
            nc.vector.memset(qb, 0.0)
            for t in range(2):
                for hh in range(4):
                    h = 4 * t + hh
                    nc.vector.tensor_copy(out=qb[32 * hh:32 * (hh + 1), t, :, h],
                                          in_=qT[32 * hh:32 * (hh + 1), t, :])
            return qb

        # ---------- qkv for self-attn ----------
        htT = make_T(ht, 'htT')
        qkv = {}
        for nm, wn, bn in (('q', 'wq_s', 'bq_s'), ('k', 'wk_s', 'bk_s'), ('v', 'wv_s', 'bv_s')):
            ps = linear_psum([htT], wn)
            qkv[nm] = const.tile([BC, D], f32, tag='qkv_' + nm, name='qkv_' + nm)
            nc.vector.tensor_add(qkv[nm], ps, vsb[bn])

        qblk_s = build_qblk(qkv['q'], 'self')

        # new-key (appended k/v) terms, all-batch
        qk = const.tile([BC, D], f32, tag='qk')
        nc.vector.tensor_mul(qk, qkv['q'], qkv['k'])
        s_new = const.tile([BC, H], f32, tag='s_new')
        nc.vector.reduce_sum(out=s_new, in_=qk.rearrange('p (g s) -> p g s', g=H), axis=AX.X)
        w_new = const.tile([BC, H], f32, tag='w_new')
        nc.scalar.activation(out=w_new, in_=s_new, func=FX.Exp, scale=SCALE)
        w_newT = const.tile([H, BC], f32, tag='w_newT')
        pe_absorb(w_new)
        transpose_128(w_newT, w_new, H)

        # ---------- attention inner loop ----------
        # scores:  sps[key, t, h] = sum_d K^T[d, key] * qblk[d, h]  (2 chunk MMs)
        # weights: wex = exp(scale * sps)  (fp8)
        # numer:   psum4[d, c, 8jb+h] += V_tile[key, d] * wex[key, h]  (V stationary)
        # denom:   psum4[h, 64+jb] += wex[key, h] * keep[key]
        scr = const.tile([128, 16], f8, tag='scr')
        wex0 = None
        if ABLATE == 'noscore':
            wex0 = const.tile([128, KT_A, H], f8, tag='wex0')
            nc.vector.memset(wex0, 0.5)

        # ---------- attention inner loop (software-pipelined) ----------
        # scores:  sps[key, t, h] = sum_d K^T[d, key] * qblk[d, h]  (2 chunk MMs)
        # weights: wex = exp(scale * sps)  (fp8, ACT)
        # pipeline: scores/exp of instance b+1 issue before V-matmuls of b,
        # so the PE never stalls on the ACT exp round-trip.
        # numer:   psum4[d, c, 8jb+h] += V_tile[key, d] * wex[key, h]  (V stationary,
        #          chunk-contiguous accumulation passes)
        # denom:   DVE tile-reduce of wex (masked for cross) + one matmul
        def attention(qblk, n_tiles, KV_dram, attT_dst, dnmix, masked):
            ksz = n_tiles * 128           # K^T bytes per chunk per partition
            voff = 2 * ksz                # V region offset
            tot = voff + n_tiles * 256
            ngroups = BC // GB
            kvps = [None] * ngroups
            psum4s = [None] * ngroups
            pend = {}
            if ABLATE:
                nc.vector.memset(attT_dst.bitcast(f32), 0.01)
                nc.vector.memset(dnmix[0], 1.0)

            kv_p = kva_p if masked else kvs_p
            def issue_dma(g):
                kvps[g] = kv_p.tile([128, GB, tot], f8, tag='kv%d' % tot, name='kvp%d' % g)
                hb = GB // 2
                for half in range(2):
                    nc.sync.dma_start(
                        out=kvps[g][:, hb * half:hb * (half + 1), :],
                        in_=KV_dram[GB * g + hb * half:GB * g + hb * (half + 1)]
                        .rearrange('b p x -> p b x'))
                if ABLATE == 'dma':
                    nc.vector.tensor_copy(out=scr[:, 0:8], in_=kvps[g][:, 0, 0:8])

            def scores_exp(bi):
                g, jb = divmod(bi, GB)
                kv = kvps[g][:, jb, :]
                if ABLATE == 'noscore':
                    pend[bi] = (wex0, kv)
                    return
                sps = sc_ps.tile([128, KT_A, H], f32, tag='scps')
                if ABLATE == 'halfs':
                    for t in range(n_tiles):
                        nc.tensor.matmul(sps[:, t, :], kv[:, 128 * t:128 * (t + 1)],
                                         qblk[:, 0, GB * g + jb, :], start=True, stop=True)
                else:
                    for t in range(n_tiles):
                        nc.tensor.matmul(sps[:, t, :], kv[:, 128 * t:128 * (t + 1)],
                                         qblk[:, 0, GB * g + jb, :], start=True, stop=False)
                        nc.tensor.matmul(sps[:, t, :], kv[:, ksz + 128 * t:ksz + 128 * (t + 1)],
                                         qblk[:, 1, GB * g + jb, :], start=False, stop=True)
                wex = wx_p.tile([128, KT_A, H], f8, tag='wex')
                nc.scalar.activation(out=wex[:, 0:n_tiles, :], in_=sps[:, 0:n_tiles, :],
                                     func=FX.Exp, scale=SCALE)
                pend[bi] = (wex, kv)

            def v_dn(bi):
                g, jb = divmod(bi, GB)
                b = bi
                wex, kv = pend.pop(bi)
                if ABLATE == 'nov':
                    nc.vector.tensor_copy(out=scr[:, 8:16], in_=wex[:, 0, 0:8])
                    return
                if jb == 0:
                    psum4s[g] = at_ps.tile([128, 72], f32, tag='at72', name='at72_%d' % g)
                psum4 = psum4s[g]
                # numerator (V stationary), chunk-contiguous accumulation passes
                for c in range(1 if ABLATE == 'halfv' else 2):
                    for t in range(n_tiles):
                        vt = kv[:, voff + 256 * t + 128 * c:voff + 256 * t + 128 * (c + 1)]
                        nc.tensor.matmul(psum4[:, 32 * c + 8 * jb:32 * c + 8 * jb + 8],
                                         vt, wex[:, t, :],
                                         start=(t == 0), stop=(t == n_tiles - 1))
                # denominator: DVE tile-reduce (masked for cross) + one matmul
                dnp = dn_p.tile([128, 8], f32, tag='dnp')
                if masked:
                    wexm = dn_p.tile([128, KT_A, 8], f32, tag='wexm')
                    nc.vector.tensor_tensor(
                        out=wexm[:, 0:n_tiles, :], in0=wex[:, 0:n_tiles, :],
                        in1=notm8[:, 0:n_tiles, b:b + 1].broadcast_to([128, n_tiles, H]),
                        op=ALU.mult)
                    red_in = wexm[:, 0:n_tiles, :]
                else:
                    red_in = wex[:, 0:n_tiles, :]
                nc.vector.reduce_sum(out=dnp, in_=red_in.rearrange('p t h -> p h t'),
                                     axis=AX.X)
                nc.tensor.matmul(psum4[0:8, 64 + jb:65 + jb], dnp, ones32,
                                 start=True, stop=True)
                if jb == GB - 1:
                    epilogue(g, psum4)

            def epilogue(g, psum4):
                # denominators + unmix copies (DVE)
                if dnmix[1] is None:
                    nc.vector.tensor_copy(out=dnmix[0][:, GB * g:GB * (g + 1)],
                                          in_=psum4[0:8, 64:68])
                else:
                    nc.vector.tensor_add(dnmix[0][:, GB * g:GB * (g + 1)],
                                         psum4[0:8, 64:68],
                                         dnmix[1][:, GB * g:GB * (g + 1)])
                pview = psum4[:, 0:64].rearrange('p (c jb h) -> p c h jb', c=2, jb=GB)
                for c in range(2):
                    for k in range(4):
                        h = 4 * c + k
                        nc.vector.tensor_copy(
                            out=attT_dst[32 * k:32 * (k + 1), c, GB * g:GB * (g + 1)],
                            in_=pview[32 * k:32 * (k + 1), c, h, :])

            issue_dma(0)
            if ngroups > 1:
                issue_dma(1)
            if ABLATE == 'dma':
                for g in range(2, ngroups):
                    issue_dma(g)
                return
            LOOK = 2   # instances of scores lookahead over the V/denominator stage
            for bi in range(BC):
                g, jb = divmod(bi, GB)
                if jb == 0:
                    if bi >= LOOK:
                        v_dn(bi - LOOK)   # before the DMA wait
                    if g + 2 < ngroups:
                        issue_dma(g + 2)
                else:
                    if bi >= LOOK:
                        v_dn(bi - LOOK)
                pe_absorb(kvps[g][0:1, jb:jb + 1, 0:1])
                scores_exp(bi)
            for bi in range(BC - LOOK, BC):
                v_dn(bi)

        def apply_inv(attT_dst, dnmix_t, tagp):
            # attT *= 1/denom, broadcast per (head, batch) via indicator matmul
            inv = const.tile([H, BC], f32, tag=tagp + '_inv', name=tagp + '_inv')
            nc.vector.reciprocal(out=inv, in_=dnmix_t)
            ips = tr_ps.tile([128, 128], f32, tag='trps')
            ipv = ips[:, 0:2 * BC].rearrange('p (c b) -> p c b', c=2)
            pe_absorb(inv)
            for c in range(2):
                nc.tensor.matmul(ipv[:, c, :], unmixE[:, c, :], inv,
                                 start=True, stop=True)
            invx = const.tile([128, 2, BC], f32, tag=tagp + '_invx', name=tagp + '_invx')
            nc.vector.tensor_copy(out=invx, in_=ipv)
            nc.vector.tensor_mul(attT_dst, attT_dst.bitcast(f32), invx)

        # ---------- self attention ----------
        attT_s = const.tile([128, 2, BC], f32r, tag='attT_s')
        dnmix_s = const.tile([H, BC], f32, tag='dnmix_s')
        attention(qblk_s, KT_S, dr['KV_cache'], attT_s, (dnmix_s, w_newT), False)

        # new-key numerator: nv = v * w_new (batch layout), transpose, add to attT
        nv = const.tile([BC, D], f32, tag='nv')
        nc.vector.tensor_tensor(out=nv.rearrange('p (g s) -> p g s', g=H),
                                in0=qkv['v'].rearrange('p (g s) -> p g s', g=H),
                                in1=w_new.unsqueeze(2).broadcast_to([BC, H, DH]),
                                op=ALU.mult)
        nvT = make_T(nv, 'nvT')
        nc.vector.tensor_add(attT_s, attT_s.bitcast(f32), nvT.bitcast(f32))
        apply_inv(attT_s, dnmix_s, 'self')

        # h1 = LN1(ht + att_self @ w0_s + b0_s)
        ps = linear_psum([attT_s], 'w0_s')
        h1p = const.tile([BC, D], f32, tag='h1p')
        nc.vector.tensor_add(h1p, ps, vsb['b0_s'])
        nc.vector.tensor_add(h1p, h1p, ht)
        h1 = const.tile([BC, D], f32, tag='h1')
        layernorm(h1, h1p, 'ln1_g', 'ln1_b', 'ln1')

        # ---------- cross attention ----------
        h1T = make_T(h1, 'h1T')
        psq = linear_psum([h1T], 'wq_a')
        qa = const.tile([BC, D], f32, tag='qa')
        nc.vector.tensor_add(qa, psq, vsb['bq_a'])
        qblk_a = build_qblk(qa, 'cross')

        attT_a = const.tile([128, 2, BC], f32r, tag='attT_a')
        dnmix_a = const.tile([H, BC], f32, tag='dnmix_a')
        attention(qblk_a, KT_A, dr['KV_att'], attT_a, (dnmix_a, None), True)
        apply_inv(attT_a, dnmix_a, 'cross')

        # h2 = LN2(h1 + att_cross @ w0_a + b0_a)
        ps2 = linear_psum([attT_a], 'w0_a')
        h2p = const.tile([BC, D], f32, tag='h2p')
        nc.vector.tensor_add(h2p, ps2, vsb['b0_a'])
        nc.vector.tensor_add(h2p, h2p, h1)
        h2 = const.tile([BC, D], f32, tag='h2')
        layernorm(h2, h2p, 'ln2_g', 'ln2_b', 'ln2')

        # ---------- MLP ----------
        h2T = make_T(h2, 'h2T')
        psm = linear_psum([h2T], 'w1')
        m1 = const.tile([BC, D], f32, tag='m1')
        nc.vector.tensor_add(m1, psm, vsb['b1'])
        m1r = const.tile([BC, D], f32, tag='m1r')
        nc.scalar.activation(out=m1r, in_=m1, func=FX.Relu, scale=1.0)
        pe_absorb(m1r)
        m1T = make_T(m1r, 'm1T')
        psm2 = linear_psum([m1T], 'w2')
        h3p = const.tile([BC, D], f32, tag='h3p')
        nc.vector.tensor_add(h3p, psm2, vsb['b2'])
        nc.vector.tensor_add(h3p, h3p, h2)
        outt = const.tile([BC, D], f32, tag='outt')
        layernorm(outt, h3p, 'ln3_g', 'ln3_b', 'ln3')
        nc.scalar.dma_start(out=out_dram[:, :], in_=outt)


_CACHE = {}


def _get_nc():
    if 'nc' not in _CACHE:
        _CACHE['nc'] = _build()
    return _CACHE['nc']


def _pack_kv(K, V):
    # K, V: [BC, n, D] float arrays (already compacted/zero-padded)
    # -> [BC, 128, 2*n + (n//128)*256] fp8
    # K^T region: col (c*n + 128*t + p), partition dd  = K[b, 128t+p, 128c+dd]
    # V region:   col (2n + 256*t + j), partition p    = V[b, 128t+p, j]
    n = K.shape[1]
    nt = n // 128
    k8 = K.astype(f8np).reshape(BC, nt, 128, 2, 128)             # [b, t, p, c, dd]
    kp = np.ascontiguousarray(k8.transpose(0, 4, 3, 1, 2))       # [b, dd, c, t, p]
    kp = kp.reshape(BC, 128, 2 * n)
    v8 = V.astype(f8np).reshape(BC, nt, 128, D)                  # [b, t, p, j]
    vp = np.ascontiguousarray(v8.transpose(0, 2, 1, 3)).reshape(BC, 128, nt * D)
    return np.concatenate([kp, vp], axis=2)


def _compact(K, V, mask):
    # keep only unmasked keys (order preserved), zero-pad to NAC
    Kc = np.zeros((BC, NAC, D), dtype=np.float32)
    Vc = np.zeros((BC, NAC, D), dtype=np.float32)
    keep = np.zeros((BC, NAC), dtype=np.float32)
    for b in range(BC):
        idx = np.nonzero(~mask[b])[0]
        cnt = len(idx)
        assert cnt <= NAC, f"unmasked count {cnt} exceeds capacity {NAC}"
        Kc[b, :cnt] = K[b, idx]
        Vc[b, :cnt] = V[b, idx]
        keep[b, :cnt] = 1.0
    return Kc, Vc, keep


def _make_in_maps(inputs):
    np_in = {k: np.asarray(v) for k, v in inputs.items()}
    ident = np.eye(128, dtype=np.float32)
    unmix_E = np.zeros((8, 2, 128), dtype=np.float32)
    for h in range(8):
        c, k = h // 4, h % 4
        unmix_E[h, c, 32 * k:32 * (k + 1)] = 1.0
    in_maps = []
    for cre in range(NCORES):
        sl = slice(cre * BC, (cre + 1) * BC)
        Kc, Vc, keep = _compact(np_in['K_att'][sl], np_in['V_att'][sl],
                                np_in['mask'][sl])
        # keep-mask in packed (p, t, b) order: slot index = 128t+p
        notm = keep.reshape(BC, KT_A, 128).transpose(2, 1, 0)
        im = {
            'h_t': np.ascontiguousarray(np_in['h_t'][sl]),
            'KV_att': _pack_kv(Kc, Vc),
            'KV_cache': _pack_kv(np_in['K_cache'][sl], np_in['V_cache'][sl]),
            'notm8': np.ascontiguousarray(notm).astype(f8np),
            'ident': ident,
            'unmix_E': unmix_E,
        }
        for n in WNAMES + BNAMES + LNAMES:
            im[n] = np.ascontiguousarray(np_in[n])
        in_maps.append(im)
    return in_maps


def run_on_device(inputs):
    nc = _get_nc()
    in_maps = _make_in_maps(inputs)
    res = bass_utils.run_bass_kernel_spmd(nc, in_maps, core_ids=list(range(NCORES)),
                                          trace=False)
    outs = [res.results[c]['out'] for c in range(NCORES)]
    return np.concatenate(outs, axis=0).astype(np.float32)


def kernel(**inputs):
    return run_on_device(inputs)


# revision 30
# speedup vs baseline: 1.0762x; 1.0329x over previous
import sys
if '/opt/trn_rl_repo' not in sys.path:
    sys.path.insert(0, '/opt/trn_rl_repo')
import numpy as np
import ml_dtypes

import concourse.bass as bass
import concourse.bacc as bacc
import concourse.tile as tile
from concourse import mybir
from concourse import bass_utils

f32 = mybir.dt.float32
f32r = mybir.dt.float32r
f8 = mybir.dt.float8e3          # e3m4: range +-15.5, 4 mantissa bits
f8np = ml_dtypes.float8_e3m4
FX = mybir.ActivationFunctionType
ALU = mybir.AluOpType
AX = mybir.AxisListType

B, D, H, DH = 256, 256, 8, 32
NCORES = 8
BC = B // NCORES          # 32 batches per core
LC = 1024                 # self-attn KV cache length
NA = 2048                 # cross-attn key count (raw)
NAC = 1152                # cross-attn keys after mask compaction (max real = 1105)
KT_S = LC // 128          # 8 key tiles (self)
KT_A = NAC // 128         # 9 key tiles (cross, compacted)
SCALE = 1.0 / float(np.sqrt(DH))
EPS = 1e-5
GB = 4                    # batches per DMA / unmix group
XS = 2 * LC + KT_S * 256      # 4096 bytes/partition per batch, self
XC = 2 * NAC + KT_A * 256     # 4608 bytes/partition per batch, cross

import os
ABLATE = os.environ.get('BASS_ABLATE', '')   # '', 'dma', 'noscore', 'nov'

WNAMES = ['wq_s', 'wk_s', 'wv_s', 'w0_s', 'wq_a', 'w0_a', 'w1', 'w2']
BNAMES = ['bq_s', 'bk_s', 'bv_s', 'b0_s', 'bq_a', 'b0_a', 'b1', 'b2']
LNAMES = ['ln1_g', 'ln1_b', 'ln2_g', 'ln2_b', 'ln3_g', 'ln3_b']


def _declare_dram(nc):
    dr = {}
    dr['h_t'] = nc.dram_tensor('h_t', [BC, 1, D], f32, kind='ExternalInput')
    # Host-packed fp8 per batch: K^T region (2 chunks x nt x 128 cols, partition=d)
    # then V region (nt tiles x 256 cols, partition=key). Masked/padded keys have
    # V rows zeroed (so they drop out of numerator and denominator).
    dr['KV_att'] = nc.dram_tensor('KV_att', [BC, 128, XC], f8, kind='ExternalInput')
    dr['KV_cache'] = nc.dram_tensor('KV_cache', [BC, 128, XS], f8, kind='ExternalInput')
    # keep-mask (1.0 = keep) per compacted key slot, fp8, for the denominator matmul
    dr['notm8'] = nc.dram_tensor('notm8', [128, KT_A, BC], f8, kind='ExternalInput')
    dr['ident'] = nc.dram_tensor('ident', [128, 128], f32, kind='ExternalInput')
    # unmix indicator: E[h, c, i] = 1 iff head h == 4c + i//32 (for inv broadcast)
    dr['unmix_E'] = nc.dram_tensor('unmix_E', [8, 2, 128], f32, kind='ExternalInput')
    for n in WNAMES:
        dr[n] = nc.dram_tensor(n, [D, D], f32r, kind='ExternalInput')
    for n in BNAMES + LNAMES:
        dr[n] = nc.dram_tensor(n, [D], f32, kind='ExternalInput')
    dr['out'] = nc.dram_tensor('out', [BC, D], f32, kind='ExternalOutput')
    return dr


def _build():
    nc = bacc.Bacc()
    dr = _declare_dram(nc)
    out = dr.pop('out')
    with tile.TileContext(nc) as tc:
        _emit(nc, tc, dr, out)
    nc.compile()
    return nc


def _emit(nc, tc, dr, out_dram):
    import contextlib
    ctx = contextlib.ExitStack()
    with ctx:
        const = ctx.enter_context(tc.tile_pool(name='const', bufs=2))
        kv_p = ctx.enter_context(tc.tile_pool(name='kv', bufs=3))
        wx_p = ctx.enter_context(tc.tile_pool(name='wx', bufs=3))
        dn_p = ctx.enter_context(tc.tile_pool(name='dn', bufs=3))
        tr_ps = ctx.enter_context(tc.tile_pool(name='trps', bufs=1, space='PSUM'))
        sc_ps = ctx.enter_context(tc.tile_pool(name='scps', bufs=3, space='PSUM'))
        at_ps = ctx.enter_context(tc.tile_pool(name='atps', bufs=2, space='PSUM'))
        ln_ps = ctx.enter_context(tc.tile_pool(name='lnps', bufs=1, space='PSUM'))
        gb_ps = ctx.enter_context(tc.tile_pool(name='gbps', bufs=1, space='PSUM'))

        garb = gb_ps.tile([1, 1], f32, tag='garb')
        last_act = [None]

        def pe_absorb(*aps):
            # PE matmul/transpose can carry only ONE sem wait in its LW slot.
            # Before a matmul whose deps span several procs, emit 1x1
            # self-matmuls so the PE observes those sems here instead.
            for a in aps:
                if a is None:
                    continue
                e = a[tuple(slice(0, 1) for _ in range(len(a.shape)))]
                if e.dtype == f32r:
                    e = e.bitcast(f32)
                nc.tensor.matmul(garb[:, :], e, e, start=True, stop=True,
                                 skip_group_check=True)

        # ---------- persistent loads ----------
        ident = const.tile([128, 128], f32, tag='ident')
        nc.gpsimd.dma_start(out=ident, in_=dr['ident'][:, :])
        pe_absorb(ident)
        ht = const.tile([BC, D], f32, tag='ht')
        nc.gpsimd.dma_start(out=ht, in_=dr['h_t'][:, 0, :])
        pe_absorb(ht)
        epst = const.tile([BC, 1], f32, tag='epst')
        nc.vector.memset(epst, EPS)
        ones8 = const.tile([128, 1], f8, tag='ones8')
        nc.vector.memset(ones8, 1.0)
        ones32 = const.tile([128, 1], f32, tag='ones32')
        nc.vector.memset(ones32, 1.0)
        unmixE = const.tile([8, 2, 128], f32, tag='unmixE')
        nc.gpsimd.dma_start(out=unmixE, in_=dr['unmix_E'][:, :, :])
        pe_absorb(unmixE)

        wsb = {}
        for n in WNAMES:
            wsb[n] = const.tile([128, 2, D], f32r, tag='w_' + n, name='w_' + n)
            nc.gpsimd.dma_start(out=wsb[n], in_=dr[n][:, :].rearrange('(t p) j -> p t j', p=128))
        vsb = {}
        for n in BNAMES + LNAMES:
            vsb[n] = const.tile([BC, D], f32, tag='v_' + n, name='v_' + n)
            nc.gpsimd.dma_start(out=vsb[n], in_=dr[n][:].unsqueeze(0).to_broadcast([BC, D]))

        notm8 = const.tile([128, KT_A, BC], f8, tag='notm8')
        nc.gpsimd.dma_start(out=notm8, in_=dr['notm8'][:, :, :])

        # ---------- helpers ----------
        def transpose_128(dst, src, cols):
            # src [rows<=128, cols<=128] SBUF f32 -> dst [cols, rows] via PE transpose
            rows = src.shape[0]
            ps = tr_ps.tile([128, 128], f32, tag='trps')
            nc.tensor.transpose(ps[0:cols, 0:rows], src, ident[0:rows, 0:rows])
            nc.vector.tensor_copy(out=dst, in_=ps[0:cols, 0:rows])

        def make_T(src_f32, tagname):
            # src [BC, D] -> [128, 2, BC] f32r transposed halves
            dstT = const.tile([128, 2, BC], f32r, tag=tagname, name=tagname)
            for t in range(2):
                transpose_128(dstT[:, t, :], src_f32[:, 128 * t:128 * (t + 1)], 128)
            return dstT

        def linear_psum(srcT_list, wname):
            # sum_t sum_s srcT.T @ W  -> psum [BC, D]
            ps = ln_ps.tile([BC, D], f32, tag='lnps')
            pe_absorb(wsb[wname])
            n_mm = 2 * len(srcT_list)
            i = 0
            for srcT in srcT_list:
                for t in range(2):
                    nc.tensor.matmul(ps[:, :], srcT[:, t, :], wsb[wname][:, t, :],
                                     start=(i == 0), stop=(i == n_mm - 1))
                    i += 1
            return ps

        def layernorm(dst, src, gname, bname, tagp):
            stats = const.tile([BC, 6], f32, tag=tagp + '_st', name=tagp + '_st')
            nc.vector.bn_stats(out=stats, in_=src)
            mv = const.tile([BC, 2], f32, tag=tagp + '_mv', name=tagp + '_mv')
            nc.vector.bn_aggr(out=mv, in_=stats)
            sd = const.tile([BC, 1], f32, tag=tagp + '_sd', name=tagp + '_sd')
            nc.scalar.activation(out=sd, in_=mv[:, 1:2], func=FX.Sqrt,
                                 bias=epst[:, :], scale=1.0)
            rstd = const.tile([BC, 1], f32, tag=tagp + '_rs', name=tagp + '_rs')
            nc.vector.reciprocal(out=rstd, in_=sd)
            nc.vector.tensor_scalar(out=dst, in0=src, scalar1=mv[:, 0:1], scalar2=rstd,
                                    op0=ALU.subtract, op1=ALU.mult)
            nc.vector.tensor_mul(dst, dst, vsb[gname])
            nc.vector.tensor_add(dst, dst, vsb[bname])

        def build_qblk(qsrc_f32, tagp):
            # -> [128, 2, BC, H] fp8, block-diagonal per head (zeros elsewhere)
            qT = make_T(qsrc_f32, tagp + '_qT')
            qb = const.tile([128, 2, BC, H], f8, tag=tagp + '_qb', name=tagp + '_qb')
            nc.vector.memset(qb, 0.0)
            for t in range(2):
                for hh in range(4):
                    h = 4 * t + hh
                    nc.vector.tensor_copy(out=qb[32 * hh:32 * (hh + 1), t, :, h],
                                          in_=qT[32 * hh:32 * (hh + 1), t, :])
            return qb

        # ---------- qkv for self-attn ----------
        htT = make_T(ht, 'htT')
        qkv = {}
        for nm, wn, bn in (('q', 'wq_s', 'bq_s'), ('k', 'wk_s', 'bk_s'), ('v', 'wv_s', 'bv_s')):
            ps = linear_psum([htT], wn)
            qkv[nm] = const.tile([BC, D], f32, tag='qkv_' + nm, name='qkv_' + nm)
            nc.vector.tensor_add(qkv[nm], ps, vsb[bn])

        qblk_s = build_qblk(qkv['q'], 'self')

        # new-key (appended k/v) terms, all-batch
        qk = const.tile([BC, D], f32, tag='qk')
        nc.vector.tensor_mul(qk, qkv['q'], qkv['k'])
        s_new = const.tile([BC, H], f32, tag='s_new')
        nc.vector.reduce_sum(out=s_new, in_=qk.rearrange('p (g s) -> p g s', g=H), axis=AX.X)
        w_new = const.tile([BC, H], f32, tag='w_new')
        nc.scalar.activation(out=w_new, in_=s_new, func=FX.Exp, scale=SCALE)
        w_newT = const.tile([H, BC], f32, tag='w_newT')
        pe_absorb(w_new)
        transpose_128(w_newT, w_new, H)

        # ---------- attention inner loop ----------
        # scores:  sps[key, t, h] = sum_d K^T[d, key] * qblk[d, h]  (2 chunk MMs)
        # weights: wex = exp(scale * sps)  (fp8)
        # numer:   psum4[d, c, 8jb+h] += V_tile[key, d] * wex[key, h]  (V stationary)
        # denom:   psum4[h, 64+jb] += wex[key, h] * keep[key]
        scr = const.tile([128, 16], f8, tag='scr')
        wex0 = None
        if ABLATE == 'noscore':
            wex0 = const.tile([128, KT_A, H], f8, tag='wex0')
            nc.vector.memset(wex0, 0.5)

        # ---------- attention inner loop (software-pipelined) ----------
        # scores:  sps[key, t, h] = sum_d K^T[d, key] * qblk[d, h]  (2 chunk MMs)
        # weights: wex = exp(scale * sps)  (fp8, ACT)
        # pipeline: scores/exp of instance b+1 issue before V-matmuls of b,
        # so the PE never stalls on the ACT exp round-trip.
        # numer:   psum4[d, c, 8jb+h] += V_tile[key, d] * wex[key, h]  (V stationary,
        #          chunk-contiguous accumulation passes)
        # denom:   DVE tile-reduce of wex (masked for cross) + one matmul
        def attention(qblk, n_tiles, KV_dram, attT_dst, dnmix, masked):
            ksz = n_tiles * 128           # K^T bytes per chunk per partition
            voff = 2 * ksz                # V region offset
            tot = voff + n_tiles * 256
            ngroups = BC // GB
            kvps = [None] * ngroups
            psum4s = [None] * ngroups
            pend = {}
            if ABLATE:
                nc.vector.memset(attT_dst.bitcast(f32), 0.01)
                nc.vector.memset(dnmix[0], 1.0)

            def issue_dma(g):
                kvps[g] = kv_p.tile([128, GB, tot], f8, tag='kv%d' % tot, name='kvp%d' % g)
                hb = GB // 2
                for half in range(2):
                    nc.sync.dma_start(
                        out=kvps[g][:, hb * half:hb * (half + 1), :],
                        in_=KV_dram[GB * g + hb * half:GB * g + hb * (half + 1)]
                        .rearrange('b p x -> p b x'))
                if ABLATE == 'dma':
                    nc.vector.tensor_copy(out=scr[:, 0:8], in_=kvps[g][:, 0, 0:8])

            def scores_exp(bi):
                g, jb = divmod(bi, GB)
                kv = kvps[g][:, jb, :]
                if ABLATE == 'noscore':
                    pend[bi] = (wex0, kv)
                    return
                sps = sc_ps.tile([128, KT_A, H], f32, tag='scps')
                if ABLATE == 'halfs':
                    for t in range(n_tiles):
                        nc.tensor.matmul(sps[:, t, :], kv[:, 128 * t:128 * (t + 1)],
                                         qblk[:, 0, GB * g + jb, :], start=True, stop=True)
                else:
                    for t in range(n_tiles):
                        nc.tensor.matmul(sps[:, t, :], kv[:, 128 * t:128 * (t + 1)],
                                         qblk[:, 0, GB * g + jb, :], start=True, stop=False)
                        nc.tensor.matmul(sps[:, t, :], kv[:, ksz + 128 * t:ksz + 128 * (t + 1)],
                                         qblk[:, 1, GB * g + jb, :], start=False, stop=True)
                wex = wx_p.tile([128, KT_A, H], f8, tag='wex')
                nc.scalar.activation(out=wex[:, 0:n_tiles, :], in_=sps[:, 0:n_tiles, :],
                                     func=FX.Exp, scale=SCALE)
                pend[bi] = (wex, kv)

            def v_dn(bi):
                g, jb = divmod(bi, GB)
                b = bi
                wex, kv = pend.pop(bi)
                if ABLATE == 'nov':
                    nc.vector.tensor_copy(out=scr[:, 8:16], in_=wex[:, 0, 0:8])
                    return
                if jb == 0:
                    psum4s[g] = at_ps.tile([128, 72], f32, tag='at72', name='at72_%d' % g)
                psum4 = psum4s[g]
                # numerator (V stationary), chunk-contiguous accumulation passes
                for c in range(1 if ABLATE == 'halfv' else 2):
                    for t in range(n_tiles):
                        vt = kv[:, voff + 256 * t + 128 * c:voff + 256 * t + 128 * (c + 1)]
                        nc.tensor.matmul(psum4[:, 32 * c + 8 * jb:32 * c + 8 * jb + 8],
                                         vt, wex[:, t, :],
                                         start=(t == 0), stop=(t == n_tiles - 1))
                # denominator: DVE tile-reduce (masked for cross) + one matmul
                dnp = dn_p.tile([128, 8], f32, tag='dnp')
                if masked:
                    wexm = dn_p.tile([128, KT_A, 8], f32, tag='wexm')
                    nc.vector.tensor_tensor(
                        out=wexm[:, 0:n_tiles, :], in0=wex[:, 0:n_tiles, :],
                        in1=notm8[:, 0:n_tiles, b:b + 1].broadcast_to([128, n_tiles, H]),
                        op=ALU.mult)
                    red_in = wexm[:, 0:n_tiles, :]
                else:
                    red_in = wex[:, 0:n_tiles, :]
                nc.vector.reduce_sum(out=dnp, in_=red_in.rearrange('p t h -> p h t'),
                                     axis=AX.X)
                nc.tensor.matmul(psum4[0:8, 64 + jb:65 + jb], dnp, ones32,
                                 start=True, stop=True)
                if jb == GB - 1:
                    epilogue(g, psum4)

            def epilogue(g, psum4):
                # denominators + unmix copies (DVE)
                if dnmix[1] is None:
                    nc.vector.tensor_copy(out=dnmix[0][:, GB * g:GB * (g + 1)],
                                          in_=psum4[0:8, 64:68])
                else:
                    nc.vector.tensor_add(dnmix[0][:, GB * g:GB * (g + 1)],
                                         psum4[0:8, 64:68],
                                         dnmix[1][:, GB * g:GB * (g + 1)])
                pview = psum4[:, 0:64].rearrange('p (c jb h) -> p c h jb', c=2, jb=GB)
                for c in range(2):
                    for k in range(4):
                        h = 4 * c + k
                        nc.vector.tensor_copy(
                            out=attT_dst[32 * k:32 * (k + 1), c, GB * g:GB * (g + 1)],
                            in_=pview[32 * k:32 * (k + 1), c, h, :])

            issue_dma(0)
            if ngroups > 1:
                issue_dma(1)
            if ABLATE == 'dma':
                for g in range(2, ngroups):
                    issue_dma(g)
                return
            LOOK = 2   # instances of scores lookahead over the V/denominator stage
            for bi in range(BC):
                g, jb = divmod(bi, GB)
                if jb == 0:
                    if bi >= LOOK:
                        v_dn(bi - LOOK)   # before the DMA wait
                    if g + 2 < ngroups:
                        issue_dma(g + 2)
                    pe_absorb(kvps[g][0:1, 0:1, 0:1])
                else:
                    if jb == GB // 2:
                        pe_absorb(kvps[g][0:1, GB // 2:GB // 2 + 1, 0:1])
                    if bi >= LOOK:
                        v_dn(bi - LOOK)
                scores_exp(bi)
            for bi in range(BC - LOOK, BC):
                v_dn(bi)

        def apply_inv(attT_dst, dnmix_t, tagp):
            # attT *= 1/denom, broadcast per (head, batch) via indicator matmul
            inv = const.tile([H, BC], f32, tag=tagp + '_inv', name=tagp + '_inv')
            nc.vector.reciprocal(out=inv, in_=dnmix_t)
            ips = tr_ps.tile([128, 128], f32, tag='trps')
            ipv = ips[:, 0:2 * BC].rearrange('p (c b) -> p c b', c=2)
            pe_absorb(inv)
            for c in range(2):
                nc.tensor.matmul(ipv[:, c, :], unmixE[:, c, :], inv,
                                 start=True, stop=True)
            invx = const.tile([128, 2, BC], f32, tag=tagp + '_invx', name=tagp + '_invx')
            nc.vector.tensor_copy(out=invx, in_=ipv)
            nc.vector.tensor_mul(attT_dst, attT_dst.bitcast(f32), invx)

        # ---------- self attention ----------
        attT_s = const.tile([128, 2, BC], f32r, tag='attT_s')
        dnmix_s = const.tile([H, BC], f32, tag='dnmix_s')
        attention(qblk_s, KT_S, dr['KV_cache'], attT_s, (dnmix_s, w_newT), False)

        # new-key numerator: nv = v * w_new (batch layout), transpose, add to attT
        nv = const.tile([BC, D], f32, tag='nv')
        nc.vector.tensor_tensor(out=nv.rearrange('p (g s) -> p g s', g=H),
                                in0=qkv['v'].rearrange('p (g s) -> p g s', g=H),
                                in1=w_new.unsqueeze(2).broadcast_to([BC, H, DH]),
                                op=ALU.mult)
        nvT = make_T(nv, 'nvT')
        nc.vector.tensor_add(attT_s, attT_s.bitcast(f32), nvT.bitcast(f32))
        apply_inv(attT_s, dnmix_s, 'self')

        # h1 = LN1(ht + att_self @ w0_s + b0_s)
        ps = linear_psum([attT_s], 'w0_s')
        h1p = const.tile([BC, D], f32, tag='h1p')
        nc.vector.tensor_add(h1p, ps, vsb['b0_s'])
        nc.vector.tensor_add(h1p, h1p, ht)
        h1 = const.tile([BC, D], f32, tag='h1')
        layernorm(h1, h1p, 'ln1_g', 'ln1_b', 'ln1')

        # ---------- cross attention ----------
        h1T = make_T(h1, 'h1T')
        psq = linear_psum([h1T], 'wq_a')
        qa = const.tile([BC, D], f32, tag='qa')
        nc.vector.tensor_add(qa, psq, vsb['bq_a'])
        qblk_a = build_qblk(qa, 'cross')

        attT_a = const.tile([128, 2, BC], f32r, tag='attT_a')
        dnmix_a = const.tile([H, BC], f32, tag='dnmix_a')
        attention(qblk_a, KT_A, dr['KV_att'], attT_a, (dnmix_a, None), True)
        apply_inv(attT_a, dnmix_a, 'cross')

        # h2 = LN2(h1 + att_cross @ w0_a + b0_a)
        ps2 = linear_psum([attT_a], 'w0_a')
        h2p = const.tile([BC, D], f32, tag='h2p')
        nc.vector.tensor_add(h2p, ps2, vsb['b0_a'])
        nc.vector.tensor_add(h2p, h2p, h1)
        h2 = const.tile([BC, D], f32, tag='h2')
        layernorm(h2, h2p, 'ln2_g', 'ln2_b', 'ln2')

        # ---------- MLP ----------
        h2T = make_T(h2, 'h2T')
        psm = linear_psum([h2T], 'w1')
        m1 = const.tile([BC, D], f32, tag='m1')
        nc.vector.tensor_add(m1, psm, vsb['b1'])
        m1r = const.tile([BC, D], f32, tag='m1r')
        nc.scalar.activation(out=m1r, in_=m1, func=FX.Relu, scale=1.0)
        pe_absorb(m1r)
        m1T = make_T(m1r, 'm1T')
        psm2 = linear_psum([m1T], 'w2')
        h3p = const.tile([BC, D], f32, tag='h3p')
        nc.vector.tensor_add(h3p, psm2, vsb['b2'])
        nc.vector.tensor_add(h3p, h3p, h2)
        outt = const.tile([BC, D], f32, tag='outt')
        layernorm(outt, h3p, 'ln3_g', 'ln3_b', 'ln3')
        nc.scalar.dma_start(out=out_dram[:, :], in_=outt)


_CACHE = {}


def _get_nc():
    if 'nc' not in _CACHE:
        _CACHE['nc'] = _build()
    return _CACHE['nc']


def _pack_kv(K, V):
    # K, V: [BC, n, D] float arrays (already compacted/zero-padded)
    # -> [BC, 128, 2*n + (n//128)*256] fp8
    # K^T region: col (c*n + 128*t + p), partition dd  = K[b, 128t+p, 128c+dd]
    # V region:   col (2n + 256*t + j), partition p    = V[b, 128t+p, j]
    n = K.shape[1]
    nt = n // 128
    k8 = K.astype(f8np).reshape(BC, nt, 128, 2, 128)             # [b, t, p, c, dd]
    kp = np.ascontiguousarray(k8.transpose(0, 4, 3, 1, 2))       # [b, dd, c, t, p]
    kp = kp.reshape(BC, 128, 2 * n)
    v8 = V.astype(f8np).reshape(BC, nt, 128, D)                  # [b, t, p, j]
    vp = np.ascontiguousarray(v8.transpose(0, 2, 1, 3)).reshape(BC, 128, nt * D)
    return np.concatenate([kp, vp], axis=2)


def _compact(K, V, mask):
    # keep only unmasked keys (order preserved), zero-pad to NAC
    Kc = np.zeros((BC, NAC, D), dtype=np.float32)
    Vc = np.zeros((BC, NAC, D), dtype=np.float32)
    keep = np.zeros((BC, NAC), dtype=np.float32)
    for b in range(BC):
        idx = np.nonzero(~mask[b])[0]
        cnt = len(idx)
        assert cnt <= NAC, f"unmasked count {cnt} exceeds capacity {NAC}"
        Kc[b, :cnt] = K[b, idx]
        Vc[b, :cnt] = V[b, idx]
        keep[b, :cnt] = 1.0
    return Kc, Vc, keep


def _make_in_maps(inputs):
    np_in = {k: np.asarray(v) for k, v in inputs.items()}
    ident = np.eye(128, dtype=np.float32)
    unmix_E = np.zeros((8, 2, 128), dtype=np.float32)
    for h in range(8):
        c, k = h // 4, h % 4
        unmix_E[h, c, 32 * k:32 * (k + 1)] = 1.0
    in_maps = []
    for cre in range(NCORES):
        sl = slice(cre * BC, (cre + 1) * BC)
        Kc, Vc, keep = _compact(np_in['K_att'][sl], np_in['V_att'][sl],
                                np_in['mask'][sl])
        # keep-mask in packed (p, t, b) order: slot index = 128t+p
        notm = keep.reshape(BC, KT_A, 128).transpose(2, 1, 0)
        im = {
            'h_t': np.ascontiguousarray(np_in['h_t'][sl]),
            'KV_att': _pack_kv(Kc, Vc),
            'KV_cache': _pack_kv(np_in['K_cache'][sl], np_in['V_cache'][sl]),
            'notm8': np.ascontiguousarray(notm).astype(f8np),
            'ident': ident,
            'unmix_E': unmix_E,
        }
        for n in WNAMES + BNAMES + LNAMES:
            im[n] = np.ascontiguousarray(np_in[n])
        in_maps.append(im)
    return in_maps


def run_on_device(inputs):
    nc = _get_nc()
    in_maps = _make_in_maps(inputs)
    res = bass_utils.run_bass_kernel_spmd(nc, in_maps, core_ids=list(range(NCORES)),
                                          trace=False)
    outs = [res.results[c]['out'] for c in range(NCORES)]
    return np.concatenate(outs, axis=0).astype(np.float32)


def kernel(**inputs):
    return run_on_device(inputs)
